# revision 21
# baseline (speedup 1.0000x reference)
"""Trainium2 Bass kernel for nn_C3S_RegularLoss.

reference:
    xr = x.reshape(B, P, D); xn = xr / ||xr||_2(axis=-1)
    s = mean_b(xn)                     # (P, D)
    corr = s @ s.T                     # (P, P)
    loss = (sum(corr) - 3*trace(corr) + 2P) / 2 * gamma

Reformulated without the corr matrix:
    sum(corr)   = || sum_p s_p ||^2
    trace(corr) = sum_p || s_p ||^2
so with S = sum_b xn (sum, not mean):
    loss = ((||sum_p S_p||^2 - 3*sum(S^2)) / B^2 + 2P) / 2 * gamma

Sharding: data-parallel over the batch dim, 8 cores x 1024 rows.
Each core computes S_partial = sum_b r_b * x_b per part via PE matmuls
(r = 1/||x_part|| as the stationary operand), one AllReduce of the
(4,2048) sums, then a tiny replicated tail computes the scalar loss.

Timeline design (from trace analysis; per-core, all times us):
  - The HBM load stream saturates (~334 GB/s); everything after it is
    the serial tail: finalize tile 7 -> cc_in DMA -> AR doorbell ->
    AR mesh (floor ~10 + rank skew) -> scalar tail -> out.
  - A tiny dummy AllReduce fires at ~8us with no data dependencies.
    It warms the TOPSP/ncfw collective stream so the real AllReduce's
    mesh starts ~1us after its doorbell instead of ~20us.  (Earlier
    versions AllReduce'd tile 0's sums mid-stream instead, but that
    trigger's completion-semaphore lane is shared round-robin with
    x-stream DMAs, so it could not fire before ~50us no matter what.)
  - ACT does the per-part sum-of-squares (2us per part).  With one DMA
    per tile, tile i's squares only start when the whole tile lands, so
    ACT runs ~10us behind the stream; unmitigated, tiles 6+7 stack
    ~17us of ACT work after the stream ends.  Fixes: tile 6 offloads
    its last part to DVE (mult+reduce), tile 7 is DMA'd
    [p0p1][p2][p3c0][p3c1] with p3's chunks squared on DVE as they
    land.  The whole finalize (chain to the AR doorbell) is then
    ~4-5us after the last byte.
  - Both ACT table sets (square, sqrt) are pre-loaded via pinned dummy
    ops so no ~1.3us table load lands mid-chain.
  - Tail after the AllReduce: load summed S as bf16 (cast in DMA),
    ones-matmuls give t = sum_p S_p in PSUM, DVE mult + PE ones-
    matmuls + a short DVE reduce give B2, ACT square+accum gives
    A = ||t||^2, and two accumulating matmuls (+1/-3) fold A - 3*B2.
    (An AllGather variant measured no faster: the mesh's skew-
    dominated first-wait governs both collective kinds here.)
"""

import os
import sys

sys.path.insert(0, "/opt/trn_rl_repo")
os.environ.setdefault("MYCRO_LOCAL_CACHE", "1")

import numpy as np

B, F = 8192, 8192
NPARTS = 4
D = F // NPARTS                 # 2048
NCORES = 8
B_CORE = B // NCORES            # 1024
TILE_P = 128
NTILES = B_CORE // TILE_P       # 8
MM_N = 512                      # moving free dim per matmul
NCHUNK = D // MM_N              # 4

_cache = {}


def _build(ncores=NCORES, collective=True):
    import concourse.bass as bass  # noqa: F401
    import concourse.mybir as mybir
    from concourse import bacc, tile
    from concourse.tile import add_dep_helper

    f32 = mybir.dt.float32
    bf16 = mybir.dt.bfloat16
    Act = mybir.ActivationFunctionType
    Alu = mybir.AluOpType
    AxX = mybir.AxisListType.X

    nc = bacc.Bacc("TRN2", num_devices=ncores, debug=False)
    x_t = nc.dram_tensor("x", [B_CORE, F], f32, kind="ExternalInput")
    g_t = nc.dram_tensor("gamma", [1, 1], f32, kind="ExternalInput")
    wsel_t = nc.dram_tensor("wsel", [NPARTS * ncores, 1 + NPARTS], bf16,
                            kind="ExternalInput")
    out_t = nc.dram_tensor("out", [1, 1], f32, kind="ExternalOutput")

    with tile.TileContext(nc) as tc:
        with tc.tile_pool(name="xp", bufs=NTILES) as xp, \
             tc.tile_pool(name="scratch", bufs=2) as scp, \
             tc.tile_pool(name="small", bufs=3) as stp, \
             tc.tile_pool(name="tail", bufs=1) as tlp, \
             tc.tile_pool(name="ps", bufs=1, space="PSUM") as psp, \
             tc.tile_pool(name="dram", bufs=1, space="DRAM") as dram:

            # PSUM accumulator: part p lives at psum partition 32*p
            # (PE col tile_position constraint), all 8 tiles accumulate.
            S_ps = psp.tile([TILE_P, D], f32, tag="accA")
            # init the junk rows (everything besides 0/32/64/96) so the
            # later full-width PSUM->SBUF copy reads defined data (the
            # simulator requires it; junk never leaves SBUF on HW).
            for bk in range(NCHUNK):
                nc.vector.memset(S_ps[:, bk * MM_N:(bk + 1) * MM_N], 0.0)
            cc_in = dram.tile([NPARTS, D], f32)
            cc_out = dram.tile([NPARTS, D], f32)
            cc_in_w = dram.tile([1, 16], f32)
            cc_out_w = dram.tile([1, 16], f32)

            # dummy warm-up AllReduce: no data deps, doorbell ~8us on
            # every rank -> ncfw/TOPSP warm long before the real one
            wsrc = tlp.tile([1, 16], f32, tag="wsrc")
            nc.vector.memset(wsrc[:], 0.0)
            nc.sync.dma_start(cc_in_w[:], wsrc[:])
            if collective:
                nc.gpsimd.collective_compute(
                    "AllReduce", Alu.add,
                    replica_groups=[list(range(ncores))],
                    ins=[cc_in_w.opt()], outs=[cc_out_w.opt()])
            else:
                nc.sync.dma_start(cc_out_w[:], cc_in_w[:])

            # pre-load both ACT table sets (square, sqrt) with dummy ops
            # so tile 0's chain isn't gated by mid-chain table loads
            warm = tlp.tile([1, 2], f32, tag="warm")
            nc.vector.memset(warm[:], 1.0)
            warm2 = tlp.tile([1, 2], f32, tag="warm2")
            nc.scalar.activation(warm2[:, 0:1], warm[:, 0:1], Act.Square)
            warm_sq = nc.scalar.activation(warm2[:, 1:2], warm[:, 1:2],
                                           Act.Rsqrt)

            prev_sqrt = None
            for i in range(NTILES):
                last = i == NTILES - 1
                # SWDGE DMA casts fp32 -> bf16 in-flight (free; PE wants
                # bf16 and the loss has ~1e3x precision headroom).
                xt = xp.tile([TILE_P, F], bf16, tag="xt")
                rows = x_t[i * TILE_P:(i + 1) * TILE_P, :]
                if last:
                    # [p0p1][p2][p3c0][p3c1]: p3's chunks square on DVE
                    # as they land, so 1/norm is ready ~1us after the
                    # stream ends
                    nc.gpsimd.dma_start(xt[:, :2 * D], rows[:, :2 * D])
                    nc.gpsimd.dma_start(xt[:, 2 * D:3 * D],
                                        rows[:, 2 * D:3 * D])
                    nc.gpsimd.dma_start(xt[:, 3 * D:3 * D + D // 2],
                                        rows[:, 3 * D:3 * D + D // 2])
                    nc.gpsimd.dma_start(xt[:, 3 * D + D // 2:],
                                        rows[:, 3 * D + D // 2:])
                else:
                    nc.gpsimd.dma_start(xt[:], rows)

                # sum-of-squares per part on ACT (square + free
                # accumulator). Keeping the big elementwise ops OFF the
                # vector engine matters mid-stream: DVE SBUF reads lock
                # GpSimd out of the port it uses for SWDGE descriptor
                # rings, which stalls the x-tile DMA stream. (Late in
                # the stream all descriptors are long emitted, so tiles
                # 6/7 can use DVE freely.)
                ss = stp.tile([TILE_P, NPARTS], f32, tag="ss")
                sqa = scp.tile([TILE_P, D], bf16, tag="sqa")
                norm = stp.tile([TILE_P, NPARTS], f32, tag="norm")
                r = stp.tile([TILE_P, NPARTS], f32, tag="r")
                r_bf = stp.tile([TILE_P, NPARTS], bf16, tag="r_bf")

                def mms_for_part(p, rbf_ap):
                    for j in range(NCHUNK):
                        nc.tensor.matmul(
                            S_ps[32 * p:32 * p + 1, j * MM_N:(j + 1) * MM_N],
                            lhsT=rbf_ap,
                            rhs=xt[:, p * D + j * MM_N:p * D + (j + 1) * MM_N],
                            start=(i == 0),
                            stop=(i == NTILES - 1),
                            tile_position=(0, 32 * p))

                if not last:
                    dve_p3 = i == NTILES - 2
                    for p in range(NPARTS - 1 if dve_p3 else NPARTS):
                        a = nc.scalar.activation(
                            sqa[:], xt[:, p * D:(p + 1) * D], Act.Square,
                            accum_out=ss[:, p:p + 1])
                        if p == 0:
                            # pin ACT order: table warm-up first, then
                            # sqrt(i-1) before squares(i), else the
                            # scheduler makes r(i-1) wait on DMA(i)
                            add_dep_helper(
                                a.ins,
                                (prev_sqrt or warm_sq).ins, sync=False,
                                reason="ACT order")
                    if dve_p3:
                        # tile 6: DVE absorbs part 3 so ACT is free for
                        # tile 7's parts the moment they land
                        p3 = NPARTS - 1
                        sq6 = scp.tile([TILE_P, D], bf16, tag="sq6")
                        nc.vector.tensor_mul(sq6[:], xt[:, p3 * D:],
                                             xt[:, p3 * D:])
                        nc.vector.tensor_reduce(ss[:, p3:p3 + 1], sq6[:],
                                                axis=AxX, op=Alu.add)
                    prev_sqrt = nc.scalar.activation(r_bf[:], ss[:],
                                                     Act.Rsqrt)
                    for p in range(NPARTS):
                        mms_for_part(p, r_bf[:, p:p + 1])
                else:
                    # tile 7: parts 0-2 on ACT (land while the stream
                    # still runs), part 3 in two DVE-squared chunks.
                    # The DVE queue is strict FIFO and the scheduler
                    # does not know the chunk DMA arrival times, so
                    # every DVE op is chain-pinned in arrival order —
                    # otherwise p0-2's reciprocals end up queued behind
                    # p3's chunk work and all matmuls slip ~5us.
                    ss3 = stp.tile([TILE_P, 3], f32, tag="ss3")
                    sq3 = scp.tile([TILE_P, D], bf16, tag="sq3")
                    p3 = NPARTS - 1
                    dve_prev = None

                    def dve_pin(inst):
                        nonlocal dve_prev
                        if dve_prev is not None:
                            add_dep_helper(inst.ins, dve_prev.ins,
                                           sync=False, reason="DVE order")
                        dve_prev = inst

                    def p3_chunk(c):
                        lo = p3 * D + c * (D // 2)
                        dve_pin(nc.vector.tensor_mul(
                            sq3[:, c * (D // 2):(c + 1) * (D // 2)],
                            xt[:, lo:lo + D // 2], xt[:, lo:lo + D // 2]))
                        dve_pin(nc.vector.tensor_reduce(
                            ss3[:, c:c + 1],
                            sq3[:, c * (D // 2):(c + 1) * (D // 2)],
                            axis=AxX, op=Alu.add))

                    pa = None
                    for p in range(NPARTS - 1):
                        a = nc.scalar.activation(
                            sqa[:], xt[:, p * D:(p + 1) * D], Act.Square,
                            accum_out=ss[:, p:p + 1])
                        add_dep_helper(
                            a.ins,
                            (pa or prev_sqrt or warm_sq).ins, sync=False,
                            reason="ACT order")
                        pa = nc.scalar.activation(r_bf[:, p:p + 1],
                                                  ss[:, p:p + 1], Act.Rsqrt)
                        mms_for_part(p, r_bf[:, p:p + 1])
                        if p == 0:
                            p3_chunk(0)
                        elif p == 1:
                            p3_chunk(1)
                    dve_pin(nc.vector.tensor_add(ss3[:, 2:3], ss3[:, 0:1],
                                                 ss3[:, 1:2]))
                    s3 = nc.scalar.activation(r_bf[:, p3:p3 + 1],
                                              ss3[:, 2:3], Act.Rsqrt)
                    if pa is not None:
                        add_dep_helper(s3.ins, pa.ins, sync=False,
                                       reason="ACT part order")
                    mms_for_part(p3, r_bf[:, p3:p3 + 1])

            g_sb = tlp.tile([1, 1], f32, tag="g_sb")
            nc.sync.dma_start(g_sb[:], g_t[:])


            # ---- partial sums -> AllGather over 8 cores ----
            # full-width PSUM->SBUF copy (cast to bf16: the exchange is
            # a pure copy and the loss term has ~1e3x precision slack)
            # split across DVE and ACT into two separate tiles — two
            # engines writing one tile get serialized by the framework
            # (rows besides 0/32/64/96 are junk zeros)
            s_lo = tlp.tile([TILE_P, D // 2], f32, tag="s_lo")
            s_hi = tlp.tile([TILE_P, D // 2], f32, tag="s_hi")
            nc.vector.tensor_copy(s_lo[:], S_ps[:, :D // 2])
            nc.scalar.copy(s_hi[:], S_ps[:, D // 2:])

            for p in range(NPARTS):
                eng = nc.sync if p % 2 == 0 else nc.scalar
                eng.dma_start(cc_in[p:p + 1, :D // 2],
                              s_lo[32 * p:32 * p + 1, :])
                eng.dma_start(cc_in[p:p + 1, D // 2:],
                              s_hi[32 * p:32 * p + 1, :])
            ar = None
            if collective:
                ar = nc.gpsimd.collective_compute(
                    "AllReduce", Alu.add,
                    replica_groups=[list(range(ncores))],
                    ins=[cc_in.opt()], outs=[cc_out.opt()])
            else:
                nc.sync.dma_start(cc_out[:], cc_in[:])

            # ---- replicated tail: loss scalar ----
            sfull = tlp.tile([NPARTS, D], bf16, tag="sfull")
            ld = nc.gpsimd.dma_start(sfull[:], cc_out[:])
            if ar is not None:
                add_dep_helper(ld.ins, ar.ins, sync=False,
                               reason="AR doorbell before load")

            ones4 = tlp.tile([NPARTS, 1], bf16, tag="ones4")
            nc.vector.memset(ones4[:], 1.0)
            onesb = tlp.tile([NPARTS, 1], bf16, tag="onesb")
            nc.vector.memset(onesb[:], 1.0)
            neg3 = tlp.tile([1, 1], f32, tag="neg3")
            nc.vector.memset(neg3[:], -3.0)
            one1 = tlp.tile([1, 1], f32, tag="one1")
            nc.vector.memset(one1[:], 1.0)

            t_ps = psp.tile([1, D], f32, tag="accA")
            for j in range(NCHUNK):
                nc.tensor.matmul(
                    t_ps[0:1, j * MM_N:(j + 1) * MM_N],
                    lhsT=ones4[:],
                    rhs=sfull[:, j * MM_N:(j + 1) * MM_N],
                    start=True, stop=True)

            sq_tail = tlp.tile([NPARTS, D], bf16, tag="sq_tail")
            nc.vector.tensor_mul(sq_tail[:], sfull[:], sfull[:])
            b2p_ps = psp.tile([1, MM_N], f32, tag="accB")
            for j in range(NCHUNK):
                nc.tensor.matmul(
                    b2p_ps[:], lhsT=onesb[:],
                    rhs=sq_tail[:, j * MM_N:(j + 1) * MM_N],
                    start=(j == 0), stop=(j == NCHUNK - 1))
            b2_sb = tlp.tile([1, 1], f32, tag="b2_sb")
            nc.vector.tensor_reduce(b2_sb[:], b2p_ps[:], axis=AxX,
                                    op=Alu.add)

            t_sq = tlp.tile([1, D], f32, tag="t_sq")
            a_sb = tlp.tile([1, 1], f32, tag="a_sb")
            nc.scalar.activation(t_sq[:], t_ps[:], Act.Square,
                                 accum_out=a_sb[:])

            ab_ps = psp.tile([1, 1], f32, tag="accB")
            nc.tensor.matmul(ab_ps[:], lhsT=one1[:], rhs=a_sb[:],
                             start=True, stop=False)
            nc.tensor.matmul(ab_ps[:], lhsT=neg3[:], rhs=b2_sb[:],
                             start=False, stop=True)

            # loss = ((A - 3*B2) / B^2 + 2P) / 2 * gamma
            l0 = tlp.tile([1, 1], f32, tag="l0")
            nc.vector.tensor_scalar(
                out=l0[:], in0=ab_ps[:],
                scalar1=1.0 / (2.0 * float(B) * float(B)),
                scalar2=float(NPARTS),
                op0=Alu.mult, op1=Alu.add)
            loss = tlp.tile([1, 1], f32, tag="loss")
            nc.vector.tensor_mul(loss[:], l0[:], g_sb[:])
            nc.sync.dma_start(out_t[:], loss[:])

    nc.compile()
    return nc


def _get_nc():
    if "nc" not in _cache:
        _cache["nc"] = _build()
    return _cache["nc"]


def kernel(x, gamma, **run_kwargs):
    from concourse import bass_utils

    x = np.ascontiguousarray(np.asarray(x, dtype=np.float32))
    gamma = np.asarray(gamma, dtype=np.float32).reshape(1, 1)
    assert x.shape == (B, F), x.shape

    nc = _get_nc()
    wsel = _wsel(NCORES)
    in_maps = [
        {"x": x[c * B_CORE:(c + 1) * B_CORE], "gamma": gamma, "wsel": wsel}
        for c in range(NCORES)
    ]
    res = bass_utils.run_bass_kernel_spmd(
        nc, in_maps, core_ids=list(range(NCORES)), **run_kwargs)
    out = np.asarray(res.results[0]["out"], dtype=np.float32).reshape(1)
    if run_kwargs.get("trace"):
        _cache["last_results"] = res
    return out


# revision 22
# speedup vs baseline: 1.0293x; 1.0293x over previous
"""Trainium2 Bass kernel for nn_C3S_RegularLoss.

reference:
    xr = x.reshape(B, P, D); xn = xr / ||xr||_2(axis=-1)
    s = mean_b(xn)                     # (P, D)
    corr = s @ s.T                     # (P, P)
    loss = (sum(corr) - 3*trace(corr) + 2P) / 2 * gamma

Reformulated without the corr matrix:
    sum(corr)   = || sum_p s_p ||^2
    trace(corr) = sum_p || s_p ||^2
so with S = sum_b xn (sum, not mean):
    loss = ((||sum_p S_p||^2 - 3*sum(S^2)) / B^2 + 2P) / 2 * gamma

Sharding: data-parallel over the batch dim, 8 cores x 1024 rows.
Each core computes S_partial = sum_b r_b * x_b per part via PE matmuls
(r = 1/||x_part|| as the stationary operand), one AllReduce of the
(4,2048) sums, then a tiny replicated tail computes the scalar loss.

Timeline design (from trace analysis; per-core, all times us):
  - The HBM load stream saturates (~334 GB/s); everything after it is
    the serial tail: finalize tile 7 -> cc_in DMA -> AR doorbell ->
    AR mesh (floor ~10 + rank skew) -> scalar tail -> out.
  - A tiny dummy AllReduce fires at ~8us with no data dependencies.
    It warms the TOPSP/ncfw collective stream so the real AllReduce's
    mesh starts ~1us after its doorbell instead of ~20us.  (Earlier
    versions AllReduce'd tile 0's sums mid-stream instead, but that
    trigger's completion-semaphore lane is shared round-robin with
    x-stream DMAs, so it could not fire before ~50us no matter what.)
  - ACT does the per-part sum-of-squares (2us per part).  With one DMA
    per tile, tile i's squares only start when the whole tile lands, so
    ACT runs ~10us behind the stream; unmitigated, tiles 6+7 stack
    ~17us of ACT work after the stream ends.  Fixes: tile 6 offloads
    its last part to DVE (mult+reduce), tile 7 is DMA'd
    [p0p1][p2][p3c0][p3c1] with p3's chunks squared on DVE as they
    land.  The whole finalize (chain to the AR doorbell) is then
    ~4-5us after the last byte.
  - Both ACT table sets (square, sqrt) are pre-loaded via pinned dummy
    ops so no ~1.3us table load lands mid-chain.
  - Tail after the AllReduce: load summed S as bf16 (cast in DMA),
    ones-matmuls give t = sum_p S_p in PSUM, DVE mult + PE ones-
    matmuls + a short DVE reduce give B2, ACT square+accum gives
    A = ||t||^2, and two accumulating matmuls (+1/-3) fold A - 3*B2.
    (An AllGather variant measured no faster: the mesh's skew-
    dominated first-wait governs both collective kinds here.)
"""

import os
import sys

sys.path.insert(0, "/opt/trn_rl_repo")
os.environ.setdefault("MYCRO_LOCAL_CACHE", "1")

import numpy as np

B, F = 8192, 8192
NPARTS = 4
D = F // NPARTS                 # 2048
NCORES = 8
B_CORE = B // NCORES            # 1024
TILE_P = 128
NTILES = B_CORE // TILE_P       # 8
MM_N = 512                      # moving free dim per matmul
NCHUNK = D // MM_N              # 4

_cache = {}


def _build(ncores=NCORES, collective=True):
    import concourse.bass as bass  # noqa: F401
    import concourse.mybir as mybir
    from concourse import bacc, tile
    from concourse.tile import add_dep_helper

    f32 = mybir.dt.float32
    bf16 = mybir.dt.bfloat16
    Act = mybir.ActivationFunctionType
    Alu = mybir.AluOpType
    AxX = mybir.AxisListType.X

    nc = bacc.Bacc("TRN2", num_devices=ncores, debug=False)
    x_t = nc.dram_tensor("x", [B_CORE, F], f32, kind="ExternalInput")
    g_t = nc.dram_tensor("gamma", [1, 1], f32, kind="ExternalInput")
    wsel_t = nc.dram_tensor("wsel", [NPARTS * ncores, 1 + NPARTS], bf16,
                            kind="ExternalInput")
    out_t = nc.dram_tensor("out", [1, 1], f32, kind="ExternalOutput")

    with tile.TileContext(nc) as tc:
        with tc.tile_pool(name="xp", bufs=NTILES) as xp, \
             tc.tile_pool(name="scratch", bufs=2) as scp, \
             tc.tile_pool(name="small", bufs=3) as stp, \
             tc.tile_pool(name="tail", bufs=1) as tlp, \
             tc.tile_pool(name="ps", bufs=1, space="PSUM") as psp, \
             tc.tile_pool(name="dram", bufs=1, space="DRAM") as dram:

            # PSUM accumulator: part p lives at psum partition 32*p
            # (PE col tile_position constraint), all 8 tiles accumulate.
            S_ps = psp.tile([TILE_P, D], f32, tag="accA")
            # init the junk rows (everything besides 0/32/64/96) so the
            # later full-width PSUM->SBUF copy reads defined data (the
            # simulator requires it; junk never leaves SBUF on HW).
            for bk in range(NCHUNK):
                nc.vector.memset(S_ps[:, bk * MM_N:(bk + 1) * MM_N], 0.0)
            cc_in = dram.tile([NPARTS, D], bf16)
            cc_out = dram.tile([NPARTS, D], bf16)
            cc_in_w = dram.tile([1, 16], f32)
            cc_out_w = dram.tile([1, 16], f32)

            # dummy warm-up AllReduce: no data deps, doorbell ~8us on
            # every rank -> ncfw/TOPSP warm long before the real one
            wsrc = tlp.tile([1, 16], f32, tag="wsrc")
            nc.vector.memset(wsrc[:], 0.0)
            nc.sync.dma_start(cc_in_w[:], wsrc[:])
            if collective:
                nc.gpsimd.collective_compute(
                    "AllReduce", Alu.add,
                    replica_groups=[list(range(ncores))],
                    ins=[cc_in_w.opt()], outs=[cc_out_w.opt()])
            else:
                nc.sync.dma_start(cc_out_w[:], cc_in_w[:])

            # pre-load both ACT table sets (square, sqrt) with dummy ops
            # so tile 0's chain isn't gated by mid-chain table loads
            warm = tlp.tile([1, 2], f32, tag="warm")
            nc.vector.memset(warm[:], 1.0)
            warm2 = tlp.tile([1, 2], f32, tag="warm2")
            nc.scalar.activation(warm2[:, 0:1], warm[:, 0:1], Act.Square)
            warm_sq = nc.scalar.activation(warm2[:, 1:2], warm[:, 1:2],
                                           Act.Rsqrt)

            prev_sqrt = None
            for i in range(NTILES):
                last = i == NTILES - 1
                # SWDGE DMA casts fp32 -> bf16 in-flight (free; PE wants
                # bf16 and the loss has ~1e3x precision headroom).
                xt = xp.tile([TILE_P, F], bf16, tag="xt")
                rows = x_t[i * TILE_P:(i + 1) * TILE_P, :]
                if last:
                    # [p0p1][p2][p3c0..c3]: p3's chunks square on DVE
                    # as they land, so 1/norm is ready ~1us after the
                    # stream ends
                    nc.gpsimd.dma_start(xt[:, :2 * D], rows[:, :2 * D])
                    nc.gpsimd.dma_start(xt[:, 2 * D:3 * D],
                                        rows[:, 2 * D:3 * D])
                    for c in range(NCHUNK):
                        lo = 3 * D + c * MM_N
                        nc.gpsimd.dma_start(xt[:, lo:lo + MM_N],
                                            rows[:, lo:lo + MM_N])
                else:
                    nc.gpsimd.dma_start(xt[:], rows)

                # sum-of-squares per part on ACT (square + free
                # accumulator). Keeping the big elementwise ops OFF the
                # vector engine matters mid-stream: DVE SBUF reads lock
                # GpSimd out of the port it uses for SWDGE descriptor
                # rings, which stalls the x-tile DMA stream. (Late in
                # the stream all descriptors are long emitted, so tiles
                # 6/7 can use DVE freely.)
                ss = stp.tile([TILE_P, NPARTS], f32, tag="ss")
                sqa = scp.tile([TILE_P, D], bf16, tag="sqa")
                norm = stp.tile([TILE_P, NPARTS], f32, tag="norm")
                r = stp.tile([TILE_P, NPARTS], f32, tag="r")
                r_bf = stp.tile([TILE_P, NPARTS], bf16, tag="r_bf")

                def mms_for_part(p, rbf_ap):
                    for j in range(NCHUNK):
                        nc.tensor.matmul(
                            S_ps[32 * p:32 * p + 1, j * MM_N:(j + 1) * MM_N],
                            lhsT=rbf_ap,
                            rhs=xt[:, p * D + j * MM_N:p * D + (j + 1) * MM_N],
                            start=(i == 0),
                            stop=(i == NTILES - 1),
                            tile_position=(0, 32 * p))

                if not last:
                    dve_p3 = i == NTILES - 2
                    for p in range(NPARTS - 1 if dve_p3 else NPARTS):
                        a = nc.scalar.activation(
                            sqa[:], xt[:, p * D:(p + 1) * D], Act.Square,
                            accum_out=ss[:, p:p + 1])
                        if p == 0:
                            # pin ACT order: table warm-up first, then
                            # sqrt(i-1) before squares(i), else the
                            # scheduler makes r(i-1) wait on DMA(i)
                            add_dep_helper(
                                a.ins,
                                (prev_sqrt or warm_sq).ins, sync=False,
                                reason="ACT order")
                    if dve_p3:
                        # tile 6: DVE absorbs part 3 so ACT is free for
                        # tile 7's parts the moment they land
                        p3 = NPARTS - 1
                        sq6 = scp.tile([TILE_P, D], bf16, tag="sq6")
                        nc.vector.tensor_mul(sq6[:], xt[:, p3 * D:],
                                             xt[:, p3 * D:])
                        nc.vector.tensor_reduce(ss[:, p3:p3 + 1], sq6[:],
                                                axis=AxX, op=Alu.add)
                    prev_sqrt = nc.scalar.activation(r_bf[:], ss[:],
                                                     Act.Rsqrt)
                    for p in range(NPARTS):
                        mms_for_part(p, r_bf[:, p:p + 1])
                else:
                    # tile 7: parts 0-2 on ACT (land while the stream
                    # still runs), part 3 in two DVE-squared chunks.
                    # The DVE queue is strict FIFO and the scheduler
                    # does not know the chunk DMA arrival times, so
                    # every DVE op is chain-pinned in arrival order —
                    # otherwise p0-2's reciprocals end up queued behind
                    # p3's chunk work and all matmuls slip ~5us.
                    ss3 = stp.tile([TILE_P, NCHUNK + 3], f32, tag="ss3")
                    sq3 = scp.tile([TILE_P, D], bf16, tag="sq3")
                    p3 = NPARTS - 1
                    dve_prev = None

                    def dve_pin(inst):
                        nonlocal dve_prev
                        if dve_prev is not None:
                            add_dep_helper(inst.ins, dve_prev.ins,
                                           sync=False, reason="DVE order")
                        dve_prev = inst

                    def p3_chunk(c):
                        lo = p3 * D + c * MM_N
                        dve_pin(nc.vector.tensor_mul(
                            sq3[:, c * MM_N:(c + 1) * MM_N],
                            xt[:, lo:lo + MM_N], xt[:, lo:lo + MM_N]))
                        dve_pin(nc.vector.tensor_reduce(
                            ss3[:, c:c + 1],
                            sq3[:, c * MM_N:(c + 1) * MM_N],
                            axis=AxX, op=Alu.add))

                    pa = None
                    for p in range(NPARTS - 1):
                        a = nc.scalar.activation(
                            sqa[:], xt[:, p * D:(p + 1) * D], Act.Square,
                            accum_out=ss[:, p:p + 1])
                        add_dep_helper(
                            a.ins,
                            (pa or prev_sqrt or warm_sq).ins, sync=False,
                            reason="ACT order")
                        pa = nc.scalar.activation(r_bf[:, p:p + 1],
                                                  ss[:, p:p + 1], Act.Rsqrt)
                        mms_for_part(p, r_bf[:, p:p + 1])
                        if p == 0:
                            p3_chunk(0)
                        elif p == 1:
                            p3_chunk(1)
                    dve_pin(nc.vector.tensor_add(ss3[:, 2:3], ss3[:, 0:1],
                                                 ss3[:, 1:2]))
                    s3 = nc.scalar.activation(r_bf[:, p3:p3 + 1],
                                              ss3[:, 2:3], Act.Rsqrt)
                    if pa is not None:
                        add_dep_helper(s3.ins, pa.ins, sync=False,
                                       reason="ACT part order")
                    mms_for_part(p3, r_bf[:, p3:p3 + 1])

            g_sb = tlp.tile([1, 1], f32, tag="g_sb")
            nc.sync.dma_start(g_sb[:], g_t[:])


            # ---- partial sums -> AllGather over 8 cores ----
            # full-width PSUM->SBUF copy (cast to bf16: the exchange is
            # a pure copy and the loss term has ~1e3x precision slack)
            # split across DVE and ACT into two separate tiles — two
            # engines writing one tile get serialized by the framework
            # (rows besides 0/32/64/96 are junk zeros)
            s_lo = tlp.tile([TILE_P, D // 2], bf16, tag="s_lo")
            s_hi = tlp.tile([TILE_P, D // 2], bf16, tag="s_hi")
            nc.vector.tensor_copy(s_lo[:], S_ps[:, :D // 2])
            nc.scalar.copy(s_hi[:], S_ps[:, D // 2:])

            for p in range(NPARTS):
                eng = nc.sync if p % 2 == 0 else nc.scalar
                eng.dma_start(cc_in[p:p + 1, :D // 2],
                              s_lo[32 * p:32 * p + 1, :])
                eng.dma_start(cc_in[p:p + 1, D // 2:],
                              s_hi[32 * p:32 * p + 1, :])
            ar = None
            if collective:
                ar = nc.gpsimd.collective_compute(
                    "AllReduce", Alu.add,
                    replica_groups=[list(range(ncores))],
                    ins=[cc_in.opt()], outs=[cc_out.opt()])
            else:
                nc.sync.dma_start(cc_out[:], cc_in[:])

            # ---- replicated tail: loss scalar ----
            sfull = tlp.tile([NPARTS, D], bf16, tag="sfull")
            ld = nc.sync.dma_start(sfull[:], cc_out[:])
            if ar is not None:
                add_dep_helper(ld.ins, ar.ins, sync=False,
                               reason="AR doorbell before load")

            ones4 = tlp.tile([NPARTS, 1], bf16, tag="ones4")
            nc.vector.memset(ones4[:], 1.0)
            onesb = tlp.tile([NPARTS, 1], bf16, tag="onesb")
            nc.vector.memset(onesb[:], 1.0)
            neg3 = tlp.tile([1, 1], f32, tag="neg3")
            nc.vector.memset(neg3[:], -3.0)
            one1 = tlp.tile([1, 1], f32, tag="one1")
            nc.vector.memset(one1[:], 1.0)

            t_ps = psp.tile([1, D], f32, tag="accA")
            for j in range(NCHUNK):
                nc.tensor.matmul(
                    t_ps[0:1, j * MM_N:(j + 1) * MM_N],
                    lhsT=ones4[:],
                    rhs=sfull[:, j * MM_N:(j + 1) * MM_N],
                    start=True, stop=True)

            sq_tail = tlp.tile([NPARTS, D], bf16, tag="sq_tail")
            nc.vector.tensor_mul(sq_tail[:], sfull[:], sfull[:])
            b2p_ps = psp.tile([1, MM_N], f32, tag="accB")
            for j in range(NCHUNK):
                nc.tensor.matmul(
                    b2p_ps[:], lhsT=onesb[:],
                    rhs=sq_tail[:, j * MM_N:(j + 1) * MM_N],
                    start=(j == 0), stop=(j == NCHUNK - 1))
            b2_sb = tlp.tile([1, 1], f32, tag="b2_sb")
            nc.vector.tensor_reduce(b2_sb[:], b2p_ps[:], axis=AxX,
                                    op=Alu.add)

            t_sq = tlp.tile([1, D], f32, tag="t_sq")
            a_sb = tlp.tile([1, 1], f32, tag="a_sb")
            nc.scalar.activation(t_sq[:], t_ps[:], Act.Square,
                                 accum_out=a_sb[:])

            ab_ps = psp.tile([1, 1], f32, tag="accB")
            nc.tensor.matmul(ab_ps[:], lhsT=one1[:], rhs=a_sb[:],
                             start=True, stop=False)
            nc.tensor.matmul(ab_ps[:], lhsT=neg3[:], rhs=b2_sb[:],
                             start=False, stop=True)

            # loss = ((A - 3*B2) / B^2 + 2P) / 2 * gamma
            l0 = tlp.tile([1, 1], f32, tag="l0")
            nc.vector.tensor_scalar(
                out=l0[:], in0=ab_ps[:],
                scalar1=1.0 / (2.0 * float(B) * float(B)),
                scalar2=float(NPARTS),
                op0=Alu.mult, op1=Alu.add)
            loss = tlp.tile([1, 1], f32, tag="loss")
            nc.vector.tensor_mul(loss[:], l0[:], g_sb[:])
            nc.sync.dma_start(out_t[:], loss[:])

    nc.compile()
    return nc


def _get_nc():
    if "nc" not in _cache:
        _cache["nc"] = _build()
    return _cache["nc"]


def kernel(x, gamma, **run_kwargs):
    from concourse import bass_utils

    x = np.ascontiguousarray(np.asarray(x, dtype=np.float32))
    gamma = np.asarray(gamma, dtype=np.float32).reshape(1, 1)
    assert x.shape == (B, F), x.shape

    nc = _get_nc()
    wsel = _wsel(NCORES)
    in_maps = [
        {"x": x[c * B_CORE:(c + 1) * B_CORE], "gamma": gamma, "wsel": wsel}
        for c in range(NCORES)
    ]
    res = bass_utils.run_bass_kernel_spmd(
        nc, in_maps, core_ids=list(range(NCORES)), **run_kwargs)
    out = np.asarray(res.results[0]["out"], dtype=np.float32).reshape(1)
    if run_kwargs.get("trace"):
        _cache["last_results"] = res
    return out
